# revision 16
# baseline (speedup 1.0000x reference)
"""CapsuleLayer dynamic-routing kernel for Trainium2 (8 NeuronCores).

Problem: inputs [B=32, I=2048, J=16], W [N=64, I=2048, D=32, J=16], routings=3.
  inputs_hat[b,n,i,d] = sum_j inputs[b,i,j] * W[n,i,d,j]
  3 rounds of routing (softmax over n, weighted sum over i, squash over d).

Strategy: shard the input-capsule axis I across the 8 cores (I_loc=256).
W is cast to bf16 on host and resides wholly in SBUF (16.75 MB), loaded
once. Each core recomputes its ihat shard from W every round; ihat never
touches DRAM. Only cross-core traffic is the [B, N, D] partial sum s,
AllReduced (256 KB) once per round.

Matmuls run in single-product bf16 (harness gate is 2e-2 rel err).
H-compute uses a block-diagonal packing: 4 capsules share one K=64
matmul with M=128 output partitions (4i x 32b), i.e. guaranteed 4x PE
utilization vs per-capsule K=16 matmuls.

On-chip layout per round, per group of 4 input capsules i:
  PE:  4x matmul [K=64=(4i,16j), M=128=(4i,32b), N=512] -> H [128, (n,d)]
  ACT: H psum -> SBUF bf16
  DVE: tmp = H*vb (bf16 2x); y = reduce_d(tmp); b += y;
       e = exp(b) (ACT, accum Z); cg2 = e/Z duplicated in pairs;
       tmp2 = H*c via pair-broadcast AP (bf16 2x, partly on GpSimd)
  PE:  s_psum += sel.T @ tmp2  (folds partition groups AND sums over i)
"""

import sys

for p in ("/opt/trn_rl_repo",):
    if p not in sys.path:
        sys.path.insert(0, p)

import ml_dtypes
import numpy as np

import concourse.bacc as bacc
import concourse.mybir as mybir
import concourse.tile as tile
from concourse.bass_utils import run_bass_kernel_spmd

# problem constants (hardcoded per harness contract)
B, N, I, D, J = 32, 64, 2048, 32, 16
R = 3  # routings
CORES = 8
I_LOC = I // CORES  # 256
ND = N * D  # 2048
EPS = 1e-7

F32 = mybir.dt.float32
BF16 = mybir.dt.bfloat16
FX = mybir.AxisListType.X
ADD = mybir.AluOpType.add
ACT = mybir.ActivationFunctionType

GROUPS = I_LOC // 4  # 64 groups of 4 capsules per round
CHUNKS = I_LOC * J // 128  # 32 row-chunks of the [(i j), *] operands


def _squash_build(nc, vbpool, smalls, kp, s4, eps_ap, scale0, out_dtype):
    """s4: [128, 2048] f32 tile holding raw s (replicated x4 on partition
    groups); real s = scale0 * s4. Returns [128, 2048] squash(s) tile."""
    # sq = sum_d (scale0*s)^2  (Square activation applies scale before func)
    sq = smalls.tile([128, N], F32, tag="sq_sq")
    s2 = kp.tile([128, ND], BF16, tag="tmp")
    nc.scalar.activation(s2[:], s4[:], ACT.Square, scale=float(scale0))
    nc.vector.tensor_reduce(
        sq[:], s2[:].rearrange("p (n d) -> p n d", d=D), axis=FX, op=ADD)
    # t = sqrt(sq + eps)
    t = smalls.tile([128, N], F32, tag="sq_t")
    nc.scalar.activation(t[:], sq[:], ACT.Sqrt, bias=eps_ap)
    # q1 = 1 + sq
    q1 = smalls.tile([128, N], F32, tag="sq_q1")
    nc.scalar.activation(q1[:], sq[:], ACT.Identity, bias=1.0)
    den = smalls.tile([128, N], F32, tag="sq_den")
    nc.vector.tensor_mul(den[:], q1[:], t[:])
    rs = smalls.tile([128, N], F32, tag="sq_rs")
    nc.vector.reciprocal(rs[:], den[:])
    # scale = sq * rs * scale0   (the last scale0 from s = scale0*raw)
    scale = smalls.tile([128, N], F32, tag="sq_scale")
    nc.vector.tensor_mul(scale[:], sq[:], rs[:])
    if scale0 != 1.0:
        nc.vector.tensor_scalar_mul(scale[:], scale[:], float(scale0))
    vb = vbpool.tile([128, ND], out_dtype, tag="sq_vb")
    nc.vector.tensor_mul(
        vb[:].rearrange("p (n d) -> p n d", d=D),
        s4[:].rearrange("p (n d) -> p n d", d=D),
        scale[:, :, None].broadcast_to([128, N, D]),
    )
    return vb


def build_kernel():
    nc = bacc.Bacc("TRN2", target_bir_lowering=False, debug=False)

    # all three staged in partition-major layout: [128, chunks * cols] so a
    # partition-slice DMA is one contiguous run per partition
    xt = nc.dram_tensor("xt", [128, CHUNKS * B], BF16, kind="ExternalInput")
    xbdt = nc.dram_tensor("xbdt", [128, CHUNKS * 128], BF16,
                          kind="ExternalInput")
    wt = nc.dram_tensor("wt", [128, CHUNKS * ND], BF16, kind="ExternalInput")
    out = nc.dram_tensor("out", [B, N, D], F32, kind="ExternalOutput")

    # collective bounce buffers (one pair per round)
    s_in = [nc.dram_tensor(f"s_in{r}", [B, ND], F32) for r in range(R)]
    s_out = [nc.dram_tensor(f"s_out{r}", [B, ND], F32, addr_space="Shared")
             for r in range(R)]

    with tile.TileContext(nc) as tc:
        with (
            tc.tile_pool(name="persist", bufs=1) as pp,
            tc.tile_pool(name="vbp", bufs=1) as vbp,
            tc.tile_pool(name="s4p", bufs=1) as s4p,
            tc.tile_pool(name="slocp", bufs=1) as slocp,
            tc.tile_pool(name="hsbp", bufs=2) as hsbp,
            tc.tile_pool(name="tmpp", bufs=1) as tmpp,
            tc.tile_pool(name="t2p", bufs=2) as t2p,
            tc.tile_pool(name="redp", bufs=1) as redp,
            tc.tile_pool(name="small", bufs=2) as sp,
            tc.tile_pool(name="psum", bufs=2, space="PSUM") as pgp,
            tc.tile_pool(name="psumB", bufs=1, space="PSUM") as psS,
        ):
            # ---- resident tiles ----
            # W, chunked rows: wsb[p, k, f] = W row 128k+p (partition-major
            # in DRAM; 16 partition-slice DMAs, one contiguous run each)
            wsb = pp.tile([128, CHUNKS, ND], BF16, tag="wsb")
            for u in range(16):
                nc.sync.dma_start(
                    wsb[u * 8:(u + 1) * 8, :, :],
                    wt[u * 8:(u + 1) * 8, :].rearrange(
                        "p (k f) -> p k f", f=ND))
            # x chunks for round-0 dense einsum: same row chunking as W
            xsb = pp.tile([128, CHUNKS, B], BF16, tag="xsb")
            nc.sync.dma_start(
                xsb[:], xt[:].rearrange("p (k b) -> p k b", b=B))
            # block-diagonal x for rounds 1-2
            xbd = pp.tile([128, CHUNKS, 128], BF16, tag="xbd")
            for u in range(4):
                nc.sync.dma_start(
                    xbd[u * 32:(u + 1) * 32, :, :],
                    xbdt[u * 32:(u + 1) * 32, :].rearrange(
                        "p (k m) -> p k m", m=128))

            # routing state: exp(b) per group, bf16 [128=(4i,32b), GROUPS, N]
            estate = pp.tile([128, GROUPS, N], BF16, tag="estate")
            eps_t = pp.tile([128, 1], F32, tag="eps")
            nc.gpsimd.memset(eps_t[:], EPS)
            # selector[p, m] = 1.0 if p % 32 == m  (partition-group fold)
            sel_i = pp.tile([128, B], mybir.dt.int32, tag="sel_i")
            nc.gpsimd.iota(sel_i[:], [[1, B]], channel_multiplier=-1)
            nc.vector.tensor_scalar(sel_i[:], sel_i[:], 31, None,
                                    op0=mybir.AluOpType.bitwise_and)
            sel = pp.tile([128, B], BF16, tag="sel")
            nc.vector.tensor_scalar(sel[:], sel_i[:], 0, None,
                                    op0=mybir.AluOpType.is_equal)

            # ---------- round 0: c uniform -> s0 = (1/N) sum_i ihat ----------
            ps0 = psS.tile([B, ND], F32, tag="ps_s")
            for k in range(CHUNKS):
                for q in range(4):
                    nc.tensor.matmul(
                        ps0[:, q * 512:(q + 1) * 512],
                        xsb[:, k, :],
                        wsb[:, k, q * 512:(q + 1) * 512],
                        start=(k == 0), stop=(k == CHUNKS - 1),
                        skip_group_check=True,
                    )
            s_loc0 = slocp.tile([B, ND], F32, tag="s_loc")
            nc.scalar.copy(s_loc0[:], ps0[:])
            nc.sync.dma_start(s_in[0][:], s_loc0[:])
            nc.gpsimd.collective_compute(
                "AllReduce", ADD,
                replica_groups=[list(range(CORES))],
                ins=[s_in[0].ap().opt()], outs=[s_out[0].ap().opt()],
            )
            s4 = s4p.tile([128, ND], F32, tag="s4")
            for g4 in range(4):
                nc.sync.dma_start(s4[g4 * 32:(g4 + 1) * 32, :], s_out[0][:])
            vb = _squash_build(nc, vbp, sp, tmpp, s4, eps_t[:], 1.0 / N, BF16)

            # ---------- rounds 1, 2 ----------
            for r in (1, 2):
                ps_s = psS.tile([B, ND], F32, tag="ps_s")
                pending = []  # (g, tmp2) awaiting fold
                post = []     # (g, hsb) awaiting softmax/tmp2

                def flush_fold(pend, last, _ps=ps_s):
                    g0, t2 = pend
                    for q in range(4):
                        nc.tensor.matmul(
                            _ps[:, q * 512:(q + 1) * 512],
                            sel[:],
                            t2[:, q * 512:(q + 1) * 512],
                            start=(g0 == 0),
                            stop=(last and q == 3),
                            skip_group_check=True,
                        )

                def stage_c(g, y, hsb, _r=r):
                    # softmax over n via running exp(b) state (|b| is O(1):
                    # no max-subtraction needed). estate[g] = prod exp(y_r).
                    esl = estate[:, g, :]
                    zz = sp.tile([128, 1], F32, tag="zz")
                    if _r == 1:
                        nc.scalar.activation(esl, y[:], ACT.Exp,
                                             accum_out=zz[:])
                    else:
                        e2 = sp.tile([128, N], BF16, tag="e2")
                        nc.scalar.activation(e2[:], y[:], ACT.Exp)
                        nc.vector.tensor_mul(esl, esl, e2[:])
                        with nc.allow_low_precision("softmax denom in f32"):
                            nc.vector.tensor_reduce(zz[:], esl, axis=FX,
                                                    op=ADD)
                    rz = sp.tile([128, 1], F32, tag="rz")
                    nc.vector.reciprocal(rz[:], zz[:])
                    # cg2[p, n, 0] = cg2[p, n, 1] = e[p, n] / Z[p]
                    cg2 = sp.tile([128, N, 2], BF16, tag="cg2")
                    nc.vector.tensor_scalar_mul(
                        cg2[:],
                        estate[:, g, :, None].broadcast_to([128, N, 2]),
                        rz[:])
                    # tmp2 = c * H  via pair-packed broadcast (keeps DVE 2x)
                    tmp2 = t2p.tile([128, ND], BF16, tag="tmp2")
                    if g % 2 == 0:
                        # offload the n-high half to GpSimd (sparingly: it
                        # contends with DVE for the shared SBUF port)
                        for h, eng in ((0, nc.vector), (1, nc.gpsimd)):
                            eng.tensor_mul(
                                tmp2[:, h * 1024:(h + 1) * 1024].rearrange(
                                    "p (n dp two) -> p n dp two",
                                    dp=16, two=2),
                                hsb[:, h * 1024:(h + 1) * 1024].rearrange(
                                    "p (n dp two) -> p n dp two",
                                    dp=16, two=2),
                                cg2[:, h * 32:(h + 1) * 32, None, :
                                    ].broadcast_to([128, 32, 16, 2]),
                            )
                    else:
                        nc.vector.tensor_mul(
                            tmp2[:].rearrange(
                                "p (n dp two) -> p n dp two", dp=16, two=2),
                            hsb[:].rearrange(
                                "p (n dp two) -> p n dp two", dp=16, two=2),
                            cg2[:, :, None, :].broadcast_to([128, N, 16, 2]),
                        )
                    pending.append((g, tmp2))

                for g in range(GROUPS):
                    half = 64 * (g % 2)
                    chunk = g // 2
                    pg0 = pgp.tile([128, ND // 2], F32, tag="pg")
                    pg1 = pgp.tile([128, ND // 2], F32, tag="pg")
                    for h, pg in ((0, pg0), (1, pg1)):
                        for q in range(2):
                            f0 = h * 1024 + q * 512
                            nc.tensor.matmul(
                                pg[:, q * 512:(q + 1) * 512],
                                xbd[half:half + 64, chunk, :],
                                wsb[half:half + 64, chunk, f0:f0 + 512],
                                start=True, stop=True,
                            )
                    hsb = hsbp.tile([128, ND], BF16, tag="hsb")
                    nc.scalar.copy(hsb[:, :1024], pg0[:])
                    nc.scalar.copy(hsb[:, 1024:], pg1[:])
                    # y = sum_d H * v   (tree: 2x-mode halving adds, then a
                    # short 1x reduce — a direct 2048-wide reduce runs 1x)
                    tmp = tmpp.tile([128, ND], BF16, tag="tmp")
                    nc.vector.tensor_mul(tmp[:], hsb[:], vb[:])
                    t3 = tmp[:].rearrange("p (n d) -> p n d", d=D)
                    tb = redp.tile([128, N, D // 2], BF16, tag="tb")
                    nc.vector.tensor_add(tb[:], t3[:, :, 0:16], t3[:, :, 16:32])
                    tct = redp.tile([128, N, D // 4], BF16, tag="tc")
                    nc.vector.tensor_add(tct[:], tb[:, :, 0:8], tb[:, :, 8:16])
                    y = sp.tile([128, N], BF16, tag="y")
                    with nc.allow_low_precision("bf16 routing logits"):
                        nc.vector.tensor_reduce(y[:], tct[:], axis=FX, op=ADD)
                    post.append((g, y, hsb))
                    if len(post) >= 2:
                        stage_c(*post.pop(0))
                    if len(pending) >= 2:
                        flush_fold(pending.pop(0), False)
                stage_c(*post.pop(0))
                flush_fold(pending.pop(0), False)
                flush_fold(pending.pop(0), True)

                s_loc = slocp.tile([B, ND], F32, tag="s_loc")
                nc.scalar.copy(s_loc[:], ps_s[:])
                nc.sync.dma_start(s_in[r][:], s_loc[:])
                nc.gpsimd.collective_compute(
                    "AllReduce", ADD,
                    replica_groups=[list(range(CORES))],
                    ins=[s_in[r].ap().opt()], outs=[s_out[r].ap().opt()],
                )
                s4 = s4p.tile([128, ND], F32, tag="s4")
                for g4 in range(4):
                    nc.sync.dma_start(s4[g4 * 32:(g4 + 1) * 32, :],
                                      s_out[r][:])
                if r < 2:
                    vb = _squash_build(nc, vbp, sp, tmpp, s4, eps_t[:], 1.0,
                                       BF16)
                else:
                    o32 = _squash_build(nc, slocp, sp, tmpp, s4, eps_t[:], 1.0,
                                        F32)
                    nc.sync.dma_start(
                        out[:].rearrange("b n d -> b (n d)"), o32[0:32, :])

    nc.compile()
    return nc


_NC_CACHE = {}


def _get_nc():
    if "nc" not in _NC_CACHE:
        _NC_CACHE["nc"] = build_kernel()
    return _NC_CACHE["nc"]


def _make_in_maps(inputs, W):
    inputs = np.ascontiguousarray(np.asarray(inputs, dtype=np.float32))
    W = np.ascontiguousarray(np.asarray(W, dtype=np.float32))
    assert inputs.shape == (B, I, J) and W.shape == (N, I, D, J)
    def pmajor(a):
        # [(k p), cols] row-major -> [p, k*cols] partition-major
        cols = a.shape[1]
        return np.ascontiguousarray(
            a.reshape(CHUNKS, 128, cols).transpose(1, 0, 2).reshape(
                128, CHUNKS * cols))

    in_maps = []
    for c in range(CORES):
        sl = slice(c * I_LOC, (c + 1) * I_LOC)
        # x_t: [(i j), b]
        x_t = inputs[:, sl, :].transpose(1, 2, 0).reshape(
            I_LOC * J, B).astype(ml_dtypes.bfloat16)
        # w_t: [(i j), (n d)] ; w_t[(i,j),(n,d)] = W[n, i, d, j]
        w_t = W[:, sl, :, :].transpose(1, 3, 0, 2).reshape(
            I_LOC * J, ND).astype(ml_dtypes.bfloat16)
        # block-diagonal x: xbd[(i,j), (i4, b)] = x_t[(i,j), b] iff i%4==i4
        xbd = np.zeros((GROUPS, 4, J, 4, B), dtype=ml_dtypes.bfloat16)
        xv = x_t.reshape(GROUPS, 4, J, B)
        for i4 in range(4):
            xbd[:, i4, :, i4, :] = xv[:, i4]
        xbd = xbd.reshape(I_LOC * J, 128)
        in_maps.append({"xt": pmajor(x_t),
                        "xbdt": pmajor(xbd),
                        "wt": pmajor(w_t)})
    return in_maps


def _ensure_ntff_hook():
    """Register the axon NTFF profile hook if the image's antenv lacks it."""
    import types

    try:
        import antenv.axon_hooks  # noqa: F401
        return
    except ImportError:
        pass
    import antenv

    if "/root/.axon_site" not in sys.path:
        sys.path.insert(0, "/root/.axon_site")
    from trn_agent_boot.trn_boot import _ntff_profile_via_ctypes

    hook = {"h": _ntff_profile_via_ctypes("/opt/axon/libaxon_pjrt.so")}
    mod = types.ModuleType("antenv.axon_hooks")
    mod.get_axon_ntff_profile_hook = lambda: hook["h"]
    mod.set_axon_ntff_profile_hook = lambda h: hook.__setitem__("h", h)
    sys.modules["antenv.axon_hooks"] = mod
    antenv.axon_hooks = mod


def run(inputs, W, trace=False):
    nc = _get_nc()
    if trace:
        _ensure_ntff_hook()
        # zero-egress container: skip the artifact upload, keep files local
        import concourse.bass_utils as bu
        bu.upload_artifacts = lambda d: d
    res = run_bass_kernel_spmd(
        nc, _make_in_maps(inputs, W), core_ids=list(range(CORES)),
        trace=trace,
    )
    return res.results[0]["out"].reshape(B, N, D), res


def kernel(inputs, W, routings=R, **_unused):
    assert int(routings) == R
    out, _ = run(inputs, W, trace=False)
    return out


# revision 20
# speedup vs baseline: 1.3155x; 1.3155x over previous
"""CapsuleLayer dynamic-routing kernel for Trainium2 (8 NeuronCores).

Problem: inputs [B=32, I=2048, J=16], W [N=64, I=2048, D=32, J=16], routings=3.
  inputs_hat[b,n,i,d] = sum_j inputs[b,i,j] * W[n,i,d,j]
  3 rounds of routing (softmax over n, weighted sum over i, squash over d).

Strategy: shard the input-capsule axis I across the 8 cores (I_loc=256).
W is cast to bf16 on host and resides wholly in SBUF (16.75 MB), loaded
once. Each core recomputes its ihat shard from W every round; ihat never
touches DRAM. Only cross-core traffic is the [B, N, D] partial sum s,
AllReduced (256 KB) once per round.

Matmuls run in single-product bf16 (harness gate is 2e-2 rel err).
H-compute uses a block-diagonal packing: 4 capsules share one K=64
matmul with M=128 output partitions (4i x 32b), i.e. guaranteed 4x PE
utilization vs per-capsule K=16 matmuls.

On-chip layout per round, per group of 4 input capsules i:
  PE:  4x matmul [K=64=(4i,16j), M=128=(4i,32b), N=512] -> H [128, (n,d)]
  ACT: H psum -> SBUF bf16
  DVE: tmp = H*vb (bf16 2x); y = reduce_d(tmp); b += y;
       e = exp(b) (ACT, accum Z); cg2 = e/Z duplicated in pairs;
       tmp2 = H*c via pair-broadcast AP (bf16 2x, partly on GpSimd)
  PE:  s_psum += sel.T @ tmp2  (folds partition groups AND sums over i)
"""

import sys

for p in ("/opt/trn_rl_repo",):
    if p not in sys.path:
        sys.path.insert(0, p)

import ml_dtypes
import numpy as np

import concourse.bacc as bacc
import concourse.mybir as mybir
import concourse.tile as tile
from concourse.bass_utils import run_bass_kernel_spmd

# problem constants (hardcoded per harness contract)
B, N, I, D, J = 32, 64, 2048, 32, 16
R = 3  # routings
CORES = 8
I_LOC = I // CORES  # 256
ND = N * D  # 2048
EPS = 1e-7

F32 = mybir.dt.float32
BF16 = mybir.dt.bfloat16
FX = mybir.AxisListType.X
ADD = mybir.AluOpType.add
ACT = mybir.ActivationFunctionType

GROUPS = I_LOC // 4  # 64 groups of 4 capsules per round
CHUNKS = I_LOC * J // 128  # 32 row-chunks of the [(i j), *] operands


def _squash_build(nc, vbpool, smalls, kp, s4, eps_ap, scale0, out_dtype):
    """s4: [128, 2048] f32 tile holding raw s (replicated x4 on partition
    groups); real s = scale0 * s4. Returns [128, 2048] squash(s) tile."""
    # sq = sum_d (scale0*s)^2  (Square activation applies scale before func)
    sq = smalls.tile([128, N], F32, tag="sq_sq")
    s2 = kp.tile([128, ND], BF16, tag="tmp")
    nc.scalar.activation(s2[:], s4[:], ACT.Square, scale=float(scale0))
    nc.vector.tensor_reduce(
        sq[:], s2[:].rearrange("p (n d) -> p n d", d=D), axis=FX, op=ADD)
    # t = sqrt(sq + eps)
    t = smalls.tile([128, N], F32, tag="sq_t")
    nc.scalar.activation(t[:], sq[:], ACT.Sqrt, bias=eps_ap)
    # q1 = 1 + sq
    q1 = smalls.tile([128, N], F32, tag="sq_q1")
    nc.scalar.activation(q1[:], sq[:], ACT.Identity, bias=1.0)
    den = smalls.tile([128, N], F32, tag="sq_den")
    nc.vector.tensor_mul(den[:], q1[:], t[:])
    rs = smalls.tile([128, N], F32, tag="sq_rs")
    nc.vector.reciprocal(rs[:], den[:])
    # scale = sq * rs * scale0   (the last scale0 from s = scale0*raw)
    scale = smalls.tile([128, N], F32, tag="sq_scale")
    nc.vector.tensor_mul(scale[:], sq[:], rs[:])
    if scale0 != 1.0:
        nc.vector.tensor_scalar_mul(scale[:], scale[:], float(scale0))
    vb = vbpool.tile([128, ND], out_dtype, tag="sq_vb")
    nc.vector.tensor_mul(
        vb[:].rearrange("p (n d) -> p n d", d=D),
        s4[:].rearrange("p (n d) -> p n d", d=D),
        scale[:, :, None].broadcast_to([128, N, D]),
    )
    return vb


def build_kernel():
    nc = bacc.Bacc("TRN2", target_bir_lowering=False, debug=False)

    # all three staged in partition-major layout: [128, chunks * cols] so a
    # partition-slice DMA is one contiguous run per partition
    xt = nc.dram_tensor("xt", [128, CHUNKS * B], BF16, kind="ExternalInput")
    xbdt = nc.dram_tensor("xbdt", [128, CHUNKS * 128], BF16,
                          kind="ExternalInput")
    wt = nc.dram_tensor("wt", [128, CHUNKS * ND], BF16, kind="ExternalInput")
    out = nc.dram_tensor("out", [B, N, D], F32, kind="ExternalOutput")

    # collective bounce buffers (one pair per round)
    s_in = [nc.dram_tensor(f"s_in{r}", [B, ND], F32) for r in range(R)]
    s_out = [nc.dram_tensor(f"s_out{r}", [B, ND], F32, addr_space="Shared")
             for r in range(R)]

    with tile.TileContext(nc) as tc:
        with (
            tc.tile_pool(name="persist", bufs=1) as pp,
            tc.tile_pool(name="vbp", bufs=1) as vbp,
            tc.tile_pool(name="s4p", bufs=1) as s4p,
            tc.tile_pool(name="slocp", bufs=1) as slocp,
            tc.tile_pool(name="hsbp", bufs=2) as hsbp,
            tc.tile_pool(name="tmpp", bufs=1) as tmpp,
            tc.tile_pool(name="t2p", bufs=2) as t2p,
            tc.tile_pool(name="redp", bufs=1) as redp,
            tc.tile_pool(name="small", bufs=2) as sp,
            tc.tile_pool(name="psum", bufs=2, space="PSUM") as pgp,
            tc.tile_pool(name="psumB", bufs=1, space="PSUM") as psS,
        ):
            # ---- resident tiles ----
            # W, chunked rows: wsb[p, k, f] = W row 128k+p (partition-major
            # in DRAM; 16 partition-slice DMAs, one contiguous run each)
            # (chunk-wise DMAs: full 128-partition width per transfer — DMA
            # throughput needs wide partition fan-out, 4KB runs/partition)
            wsb = pp.tile([128, CHUNKS, ND], BF16, tag="wsb")
            for k in range(CHUNKS):
                nc.sync.dma_start(
                    wsb[:, k, :], wt[:, k * ND:(k + 1) * ND])
            # x chunks for round-0 dense einsum: same row chunking as W
            xsb = pp.tile([128, CHUNKS, B], BF16, tag="xsb")
            nc.sync.dma_start(
                xsb[:], xt[:].rearrange("p (k b) -> p k b", b=B))
            # block-diagonal x for rounds 1-2
            xbd = pp.tile([128, CHUNKS, 128], BF16, tag="xbd")
            for u in range(4):
                nc.sync.dma_start(
                    xbd[:, u * 8:(u + 1) * 8, :],
                    xbdt[:, u * 1024:(u + 1) * 1024].rearrange(
                        "p (k m) -> p k m", m=128))

            # routing state: exp(b) per group, bf16 [128=(4i,32b), GROUPS, N]
            estate = pp.tile([128, GROUPS, N], BF16, tag="estate")
            eps_t = pp.tile([128, 1], F32, tag="eps")
            nc.gpsimd.memset(eps_t[:], EPS)
            # selector[p, m] = 1.0 if p % 32 == m  (partition-group fold)
            sel_i = pp.tile([128, B], mybir.dt.int32, tag="sel_i")
            nc.gpsimd.iota(sel_i[:], [[1, B]], channel_multiplier=-1)
            nc.vector.tensor_scalar(sel_i[:], sel_i[:], 31, None,
                                    op0=mybir.AluOpType.bitwise_and)
            sel = pp.tile([128, B], BF16, tag="sel")
            nc.vector.tensor_scalar(sel[:], sel_i[:], 0, None,
                                    op0=mybir.AluOpType.is_equal)

            # ---------- round 0: c uniform -> s0 = (1/N) sum_i ihat ----------
            ps0 = psS.tile([B, ND], F32, tag="ps_s")
            for k in range(CHUNKS):
                for q in range(4):
                    nc.tensor.matmul(
                        ps0[:, q * 512:(q + 1) * 512],
                        xsb[:, k, :],
                        wsb[:, k, q * 512:(q + 1) * 512],
                        start=(k == 0), stop=(k == CHUNKS - 1),
                        skip_group_check=True,
                    )
            s_loc0 = slocp.tile([B, ND], F32, tag="s_loc")
            nc.scalar.copy(s_loc0[:], ps0[:])
            nc.sync.dma_start(s_in[0][:], s_loc0[:])
            nc.gpsimd.collective_compute(
                "AllReduce", ADD,
                replica_groups=[list(range(CORES))],
                ins=[s_in[0].ap().opt()], outs=[s_out[0].ap().opt()],
            )
            s4 = s4p.tile([128, ND], F32, tag="s4")
            for g4 in range(4):
                nc.sync.dma_start(s4[g4 * 32:(g4 + 1) * 32, :], s_out[0][:])
            vb = _squash_build(nc, vbp, sp, tmpp, s4, eps_t[:], 1.0 / N, BF16)

            # ---------- rounds 1, 2 ----------
            for r in (1, 2):
                ps_s = psS.tile([B, ND], F32, tag="ps_s")
                pending = []  # (g, tmp2) awaiting fold
                post = []     # (g, hsb) awaiting softmax/tmp2

                def flush_fold(pend, last, _ps=ps_s):
                    g0, t2 = pend
                    for q in range(4):
                        nc.tensor.matmul(
                            _ps[:, q * 512:(q + 1) * 512],
                            sel[:],
                            t2[:, q * 512:(q + 1) * 512],
                            start=(g0 == 0),
                            stop=(last and q == 3),
                            skip_group_check=True,
                        )

                def stage_c(g, y, hsb, _r=r):
                    # softmax over n via running exp(b) state (|b| is O(1):
                    # no max-subtraction needed). estate[g] = prod exp(y_r).
                    esl = estate[:, g, :]
                    zz = sp.tile([128, 1], F32, tag="zz")
                    if _r == 1:
                        nc.scalar.activation(esl, y[:], ACT.Exp,
                                             accum_out=zz[:])
                    else:
                        e2 = sp.tile([128, N], BF16, tag="e2")
                        nc.scalar.activation(e2[:], y[:], ACT.Exp)
                        # GpSimd: tiny op, keeps it off the busy DVE
                        nc.gpsimd.tensor_mul(esl, esl, e2[:])
                        with nc.allow_low_precision("softmax denom in f32"):
                            nc.vector.tensor_reduce(zz[:], esl, axis=FX,
                                                    op=ADD)
                    rz = sp.tile([128, 1], F32, tag="rz")
                    nc.vector.reciprocal(rz[:], zz[:])
                    # cg2[p, n, 0] = cg2[p, n, 1] = e[p, n] / Z[p]
                    # (ScalarE: activation Copy with per-partition scale)
                    cg2 = sp.tile([128, N, 2], BF16, tag="cg2")
                    nc.scalar.activation(
                        cg2[:],
                        estate[:, g, :, None].broadcast_to([128, N, 2]),
                        ACT.Copy, scale=rz[:])
                    # tmp2 = c * H  via pair-packed broadcast (keeps DVE 2x)
                    tmp2 = t2p.tile([128, ND], BF16, tag="tmp2")
                    nc.vector.tensor_mul(
                        tmp2[:].rearrange(
                            "p (n dp two) -> p n dp two", dp=16, two=2),
                        hsb[:].rearrange(
                            "p (n dp two) -> p n dp two", dp=16, two=2),
                        cg2[:, :, None, :].broadcast_to([128, N, 16, 2]),
                    )
                    pending.append((g, tmp2))

                for g in range(GROUPS):
                    half = 64 * (g % 2)
                    chunk = g // 2
                    pg0 = pgp.tile([128, ND // 2], F32, tag="pg")
                    pg1 = pgp.tile([128, ND // 2], F32, tag="pg")
                    for h, pg in ((0, pg0), (1, pg1)):
                        for q in range(2):
                            f0 = h * 1024 + q * 512
                            nc.tensor.matmul(
                                pg[:, q * 512:(q + 1) * 512],
                                xbd[half:half + 64, chunk, :],
                                wsb[half:half + 64, chunk, f0:f0 + 512],
                                start=True, stop=True,
                            )
                    hsb = hsbp.tile([128, ND], BF16, tag="hsb")
                    nc.scalar.copy(hsb[:, :1024], pg0[:])
                    nc.scalar.copy(hsb[:, 1024:], pg1[:])
                    # y = sum_d H * v   (tree: 2x-mode halving adds, then a
                    # short 1x reduce — a direct 2048-wide reduce runs 1x)
                    tmp = tmpp.tile([128, ND], BF16, tag="tmp")
                    nc.vector.tensor_mul(tmp[:], hsb[:], vb[:])
                    t3 = tmp[:].rearrange("p (n d) -> p n d", d=D)
                    tb = redp.tile([128, N, D // 2], BF16, tag="tb")
                    nc.vector.tensor_add(tb[:], t3[:, :, 0:16], t3[:, :, 16:32])
                    y = sp.tile([128, N], BF16, tag="y")
                    with nc.allow_low_precision("bf16 routing logits"):
                        nc.vector.tensor_reduce(y[:], tb[:], axis=FX, op=ADD)
                    post.append((g, y, hsb))
                    if len(post) >= 2:
                        stage_c(*post.pop(0))
                    if len(pending) >= 2:
                        flush_fold(pending.pop(0), False)
                stage_c(*post.pop(0))
                flush_fold(pending.pop(0), False)
                flush_fold(pending.pop(0), True)

                s_loc = slocp.tile([B, ND], F32, tag="s_loc")
                nc.scalar.copy(s_loc[:], ps_s[:])
                nc.sync.dma_start(s_in[r][:], s_loc[:])
                nc.gpsimd.collective_compute(
                    "AllReduce", ADD,
                    replica_groups=[list(range(CORES))],
                    ins=[s_in[r].ap().opt()], outs=[s_out[r].ap().opt()],
                )
                s4 = s4p.tile([128, ND], F32, tag="s4")
                for g4 in range(4):
                    nc.sync.dma_start(s4[g4 * 32:(g4 + 1) * 32, :],
                                      s_out[r][:])
                if r < 2:
                    vb = _squash_build(nc, vbp, sp, tmpp, s4, eps_t[:], 1.0,
                                       BF16)
                else:
                    o32 = _squash_build(nc, slocp, sp, tmpp, s4, eps_t[:], 1.0,
                                        F32)
                    nc.sync.dma_start(
                        out[:].rearrange("b n d -> b (n d)"), o32[0:32, :])

    nc.compile()
    return nc


_NC_CACHE = {}


def _get_nc():
    if "nc" not in _NC_CACHE:
        _NC_CACHE["nc"] = build_kernel()
    return _NC_CACHE["nc"]


def _make_in_maps(inputs, W):
    inputs = np.ascontiguousarray(np.asarray(inputs, dtype=np.float32))
    W = np.ascontiguousarray(np.asarray(W, dtype=np.float32))
    assert inputs.shape == (B, I, J) and W.shape == (N, I, D, J)
    def pmajor(a):
        # [(k p), cols] row-major -> [p, k*cols] partition-major
        cols = a.shape[1]
        return np.ascontiguousarray(
            a.reshape(CHUNKS, 128, cols).transpose(1, 0, 2).reshape(
                128, CHUNKS * cols))

    in_maps = []
    for c in range(CORES):
        sl = slice(c * I_LOC, (c + 1) * I_LOC)
        # x_t: [(i j), b]
        x_t = inputs[:, sl, :].transpose(1, 2, 0).reshape(
            I_LOC * J, B).astype(ml_dtypes.bfloat16)
        # w_t: [(i j), (n d)] ; w_t[(i,j),(n,d)] = W[n, i, d, j]
        w_t = W[:, sl, :, :].transpose(1, 3, 0, 2).reshape(
            I_LOC * J, ND).astype(ml_dtypes.bfloat16)
        # block-diagonal x: xbd[(i,j), (i4, b)] = x_t[(i,j), b] iff i%4==i4
        xbd = np.zeros((GROUPS, 4, J, 4, B), dtype=ml_dtypes.bfloat16)
        xv = x_t.reshape(GROUPS, 4, J, B)
        for i4 in range(4):
            xbd[:, i4, :, i4, :] = xv[:, i4]
        xbd = xbd.reshape(I_LOC * J, 128)
        in_maps.append({"xt": pmajor(x_t),
                        "xbdt": pmajor(xbd),
                        "wt": pmajor(w_t)})
    return in_maps


def _ensure_ntff_hook():
    """Register the axon NTFF profile hook if the image's antenv lacks it."""
    import types

    try:
        import antenv.axon_hooks  # noqa: F401
        return
    except ImportError:
        pass
    import antenv

    if "/root/.axon_site" not in sys.path:
        sys.path.insert(0, "/root/.axon_site")
    from trn_agent_boot.trn_boot import _ntff_profile_via_ctypes

    hook = {"h": _ntff_profile_via_ctypes("/opt/axon/libaxon_pjrt.so")}
    mod = types.ModuleType("antenv.axon_hooks")
    mod.get_axon_ntff_profile_hook = lambda: hook["h"]
    mod.set_axon_ntff_profile_hook = lambda h: hook.__setitem__("h", h)
    sys.modules["antenv.axon_hooks"] = mod
    antenv.axon_hooks = mod


def run(inputs, W, trace=False):
    nc = _get_nc()
    if trace:
        _ensure_ntff_hook()
        # zero-egress container: skip the artifact upload, keep files local
        import concourse.bass_utils as bu
        bu.upload_artifacts = lambda d: d
    res = run_bass_kernel_spmd(
        nc, _make_in_maps(inputs, W), core_ids=list(range(CORES)),
        trace=trace,
    )
    return res.results[0]["out"].reshape(B, N, D), res


def kernel(inputs, W, routings=R, **_unused):
    assert int(routings) == R
    out, _ = run(inputs, W, trace=False)
    return out


# revision 31
# speedup vs baseline: 1.4918x; 1.1340x over previous
"""CapsuleLayer dynamic-routing kernel for Trainium2 (8 NeuronCores).

Problem: inputs [B=32, I=2048, J=16], W [N=64, I=2048, D=32, J=16], routings=3.
  inputs_hat[b,n,i,d] = sum_j inputs[b,i,j] * W[n,i,d,j]
  3 rounds of routing (softmax over n, weighted sum over i, squash over d).

Strategy: shard the input-capsule axis I across the 8 cores (I_loc=256).
W is cast to bf16 on host and resides wholly in SBUF (16.75 MB), loaded
once. Each core recomputes its ihat shard from W every round; ihat never
touches DRAM. Only cross-core traffic is the [B, N, D] partial sum s,
AllReduced (256 KB) once per round.

Matmuls run in single-product bf16 (harness gate is 2e-2 rel err).
H-compute uses a block-diagonal packing: 4 capsules share one K=64
matmul with M=128 output partitions (4i x 32b), i.e. guaranteed 4x PE
utilization vs per-capsule K=16 matmuls.

On-chip layout per round, per group of 4 input capsules i:
  PE:  4x matmul [K=64=(4i,16j), M=128=(4i,32b), N=512] -> H [128, (n,d)]
  ACT: H psum -> SBUF bf16
  DVE: tmp = H*vb (bf16 2x); y = reduce_d(tmp); b += y;
       e = exp(b) (ACT, accum Z); cg2 = e/Z duplicated in pairs;
       tmp2 = H*c via pair-broadcast AP (bf16 2x, partly on GpSimd)
  PE:  s_psum += sel.T @ tmp2  (folds partition groups AND sums over i)
"""

import sys

for p in ("/opt/trn_rl_repo",):
    if p not in sys.path:
        sys.path.insert(0, p)

import ml_dtypes
import numpy as np

import concourse.bacc as bacc
import concourse.mybir as mybir
import concourse.tile as tile
from concourse.bass_utils import run_bass_kernel_spmd

# problem constants (hardcoded per harness contract)
B, N, I, D, J = 32, 64, 2048, 32, 16
R = 3  # routings
CORES = 8
I_LOC = I // CORES  # 256
ND = N * D  # 2048
EPS = 1e-7

F32 = mybir.dt.float32
BF16 = mybir.dt.bfloat16
FX = mybir.AxisListType.X
ADD = mybir.AluOpType.add
ACT = mybir.ActivationFunctionType

GROUPS = I_LOC // 4  # 64 groups of 4 capsules per round
CHUNKS = I_LOC * J // 128  # 32 row-chunks of the [(i j), *] operands


def _squash_build(nc, vbpool, smalls, kp, s4, eps_ap, scale0, out_dtype,
                  rows=128, tag="sq_vb"):
    """s4: [128, 2048] f32 tile holding raw s (replicated x4 on partition
    groups); real s = scale0 * s4. Returns [rows, 2048] squash(s) tile."""
    # sq = sum_d (scale0*s)^2  (Square activation applies scale before func)
    sq = smalls.tile([128, N], F32, tag="sq_sq")
    s2 = kp.tile([128, ND], BF16, tag="tmp")
    nc.scalar.activation(s2[:], s4[:], ACT.Square, scale=float(scale0))
    nc.vector.tensor_reduce(
        sq[:], s2[:].rearrange("p (n d) -> p n d", d=D), axis=FX, op=ADD)
    # t = sqrt(sq + eps)
    t = smalls.tile([128, N], F32, tag="sq_t")
    nc.scalar.activation(t[:], sq[:], ACT.Sqrt, bias=eps_ap)
    # q1 = 1 + sq
    q1 = smalls.tile([128, N], F32, tag="sq_q1")
    nc.scalar.activation(q1[:], sq[:], ACT.Identity, bias=1.0)
    den = smalls.tile([128, N], F32, tag="sq_den")
    nc.vector.tensor_mul(den[:], q1[:], t[:])
    rs = smalls.tile([128, N], F32, tag="sq_rs")
    nc.vector.reciprocal(rs[:], den[:])
    # scale = sq * rs * scale0   (the last scale0 from s = scale0*raw)
    scale = smalls.tile([128, N], F32, tag="sq_scale")
    nc.vector.tensor_mul(scale[:], sq[:], rs[:])
    if scale0 != 1.0:
        nc.vector.tensor_scalar_mul(scale[:], scale[:], float(scale0))
    vb = vbpool.tile([rows, ND], out_dtype, tag=tag)
    nc.vector.tensor_mul(
        vb[:].rearrange("p (n d) -> p n d", d=D),
        s4[0:rows].rearrange("p (n d) -> p n d", d=D),
        scale[0:rows, :, None].broadcast_to([rows, N, D]),
    )
    return vb


def build_kernel():
    nc = bacc.Bacc("TRN2", target_bir_lowering=False, debug=False)

    # all three staged in partition-major layout: [128, chunks * cols] so a
    # partition-slice DMA is one contiguous run per partition
    xt = nc.dram_tensor("xt", [128, CHUNKS * B], BF16, kind="ExternalInput")
    xbdt = nc.dram_tensor("xbdt", [128, CHUNKS * 128], BF16,
                          kind="ExternalInput")
    wt = nc.dram_tensor("wt", [128, CHUNKS * ND], BF16, kind="ExternalInput")
    out = nc.dram_tensor("out", [B, N, D], F32, kind="ExternalOutput")

    # collective bounce buffers (one pair per round), stored in the fold's
    # col-tiled layout: row 32q+b holds s[b, 512q:512(q+1)]
    s_in = [nc.dram_tensor(f"s_in{r}", [128, 512], F32) for r in range(R)]
    s_out = [nc.dram_tensor(f"s_out{r}", [128, 512], F32, addr_space="Shared")
             for r in range(R)]

    with tile.TileContext(nc) as tc:
        with (
            tc.tile_pool(name="persist", bufs=1) as pp,
            tc.tile_pool(name="vbp", bufs=1) as vbp,
            tc.tile_pool(name="s4p", bufs=1) as s4p,
            tc.tile_pool(name="slocp", bufs=1) as slocp,
            tc.tile_pool(name="hsbp", bufs=3) as hsbp,
            tc.tile_pool(name="tmpp", bufs=1) as tmpp,
            tc.tile_pool(name="t2p", bufs=2) as t2p,
            tc.tile_pool(name="redp", bufs=1) as redp,
            tc.tile_pool(name="small", bufs=2) as sp,
            tc.tile_pool(name="psum", bufs=3, space="PSUM") as pgp,
            tc.tile_pool(name="psumB", bufs=1, space="PSUM") as psS,
        ):
            # ---- resident tiles ----
            # W, chunked rows: wsb[p, k, f] = W row 128k+p (partition-major
            # in DRAM; 16 partition-slice DMAs, one contiguous run each)
            # (chunk-wise DMAs: full 128-partition width per transfer — DMA
            # throughput needs wide partition fan-out, 4KB runs/partition)
            wsb = pp.tile([128, CHUNKS, ND], BF16, tag="wsb")
            for k in range(CHUNKS):
                nc.sync.dma_start(
                    wsb[:, k, :], wt[:, k * ND:(k + 1) * ND])
            # x chunks for round-0 dense einsum: same row chunking as W
            xsb = pp.tile([128, CHUNKS, B], BF16, tag="xsb")
            nc.sync.dma_start(
                xsb[:], xt[:].rearrange("p (k b) -> p k b", b=B))
            # block-diagonal x for rounds 1-2
            xbd = pp.tile([128, CHUNKS, 128], BF16, tag="xbd")
            for u in range(4):
                nc.sync.dma_start(
                    xbd[:, u * 8:(u + 1) * 8, :],
                    xbdt[:, u * 1024:(u + 1) * 1024].rearrange(
                        "p (k m) -> p k m", m=128))

            # routing state: exp(b) per group, bf16 [128=(4i,32b), GROUPS, N]
            estate = pp.tile([128, GROUPS, N], BF16, tag="estate")
            eps_t = pp.tile([128, 1], F32, tag="eps")
            nc.gpsimd.memset(eps_t[:], EPS)
            # selector[p, m] = 1.0 if p % 32 == m  (partition-group fold)
            sel_i = pp.tile([128, B], mybir.dt.int32, tag="sel_i")
            nc.gpsimd.iota(sel_i[:], [[1, B]], channel_multiplier=-1)
            nc.vector.tensor_scalar(sel_i[:], sel_i[:], 31, None,
                                    op0=mybir.AluOpType.bitwise_and)
            sel = pp.tile([128, B], BF16, tag="sel")
            nc.vector.tensor_scalar(sel[:], sel_i[:], 0, None,
                                    op0=mybir.AluOpType.is_equal)

            # ---------- round 0: c uniform -> s0 = (1/N) sum_i ihat ----------
            # quarter q of (n,d) accumulates on partitions 32q..32q+32
            # (4-way col-tiled: one PSUM bank, concurrent quarter streams)
            ps0 = psS.tile([128, 512], F32, tag="ps_s")
            for k in range(CHUNKS):
                for q in range(4):
                    nc.tensor.matmul(
                        ps0[32 * q:32 * (q + 1), :],
                        xsb[:, k, :],
                        wsb[:, k, q * 512:(q + 1) * 512],
                        start=(k == 0), stop=(k == CHUNKS - 1),
                        tile_position=(0, 32 * q),
                        skip_group_check=True,
                    )
            s_loc0 = slocp.tile([128, 512], F32, tag="s_loc")
            nc.scalar.copy(s_loc0[:], ps0[:])
            nc.sync.dma_start(s_in[0][:], s_loc0[:])
            nc.gpsimd.collective_compute(
                "AllReduce", ADD,
                replica_groups=[list(range(CORES))],
                ins=[s_in[0].ap().opt()], outs=[s_out[0].ap().opt()],
            )
            s4 = s4p.tile([128, ND], F32, tag="s4")
            for g4 in range(4):
                nc.sync.dma_start(
                    s4[g4 * 32:(g4 + 1) * 32, :].rearrange(
                        "b (q f) -> b q f", f=512),
                    s_out[0][:].rearrange("(q b) f -> b q f", q=4))
            vb = _squash_build(nc, vbp, sp, tmpp, s4, eps_t[:], 1.0 / N, BF16)

            # ---------- rounds 1, 2 ----------
            for r in (1, 2):
                ps_s = psS.tile([128, 512], F32, tag="ps_s")
                pending = []  # (g, tmp2) awaiting fold
                post = []     # (g, hsb) awaiting softmax/tmp2

                def flush_fold(pend, last, _ps=ps_s):
                    g0, t2 = pend
                    for q in range(4):
                        nc.tensor.matmul(
                            _ps[32 * q:32 * (q + 1), :],
                            sel[:],
                            t2[:, q * 512:(q + 1) * 512],
                            start=(g0 == 0),
                            stop=(last and q == 3),
                            tile_position=(0, 32 * q),
                            skip_group_check=True,
                        )

                def stage_c(g, y, hsb, _r=r):
                    # softmax over n via running exp(b) state (|b| is O(1):
                    # no max-subtraction needed). estate[g] = prod exp(y_r).
                    esl = estate[:, g, :]
                    zz = sp.tile([128, 1], F32, tag="zz")
                    if _r == 1:
                        nc.scalar.activation(esl, y[:], ACT.Exp,
                                             accum_out=zz[:])
                    else:
                        e2 = sp.tile([128, N], BF16, tag="e2")
                        nc.scalar.activation(e2[:], y[:], ACT.Exp)
                        # GpSimd: tiny op, keeps it off the busy DVE
                        nc.gpsimd.tensor_mul(esl, esl, e2[:])
                        with nc.allow_low_precision("softmax denom in f32"):
                            nc.vector.tensor_reduce(zz[:], esl, axis=FX,
                                                    op=ADD)
                    rz = sp.tile([128, 1], F32, tag="rz")
                    nc.vector.reciprocal(rz[:], zz[:])
                    # cg2[p, n, 0] = cg2[p, n, 1] = e[p, n] / Z[p]
                    # (ScalarE: activation Copy with per-partition scale)
                    cg2 = sp.tile([128, N, 2], BF16, tag="cg2")
                    nc.scalar.activation(
                        cg2[:],
                        estate[:, g, :, None].broadcast_to([128, N, 2]),
                        ACT.Copy, scale=rz[:])
                    # tmp2 = c * H  via pair-packed broadcast (keeps DVE 2x)
                    tmp2 = t2p.tile([128, ND], BF16, tag="tmp2")
                    nc.vector.tensor_mul(
                        tmp2[:].rearrange(
                            "p (n dp two) -> p n dp two", dp=16, two=2),
                        hsb[:].rearrange(
                            "p (n dp two) -> p n dp two", dp=16, two=2),
                        cg2[:, :, None, :].broadcast_to([128, N, 16, 2]),
                    )
                    pending.append((g, tmp2))

                for g in range(GROUPS):
                    half = 64 * (g % 2)
                    chunk = g // 2
                    pg0 = pgp.tile([128, ND // 2], F32, tag="pg")
                    pg1 = pgp.tile([128, ND // 2], F32, tag="pg")
                    for h, pg in ((0, pg0), (1, pg1)):
                        for q in range(2):
                            f0 = h * 1024 + q * 512
                            nc.tensor.matmul(
                                pg[:, q * 512:(q + 1) * 512],
                                xbd[half:half + 64, chunk, :],
                                wsb[half:half + 64, chunk, f0:f0 + 512],
                                start=True, stop=True,
                            )
                    hsb = hsbp.tile([128, ND], BF16, tag="hsb")
                    nc.scalar.copy(hsb[:, :1024], pg0[:])
                    nc.scalar.copy(hsb[:, 1024:], pg1[:])
                    # y = sum_d H * v   (tree: 2x-mode halving adds, then a
                    # short 1x reduce — a direct 2048-wide reduce runs 1x)
                    tmp = tmpp.tile([128, ND], BF16, tag="tmp")
                    nc.vector.tensor_mul(tmp[:], hsb[:], vb[:])
                    t3 = tmp[:].rearrange("p (n d) -> p n d", d=D)
                    tb = redp.tile([128, N, D // 2], BF16, tag="tb")
                    nc.vector.tensor_add(tb[:], t3[:, :, 0:16], t3[:, :, 16:32])
                    tct = redp.tile([128, N, D // 4], BF16, tag="tc")
                    nc.vector.tensor_add(tct[:], tb[:, :, 0:8], tb[:, :, 8:16])
                    y = sp.tile([128, N], BF16, tag="y")
                    with nc.allow_low_precision("bf16 routing logits"):
                        nc.vector.tensor_reduce(y[:], tct[:], axis=FX, op=ADD)
                    post.append((g, y, hsb))
                    if len(post) >= 2:
                        stage_c(*post.pop(0))
                    if len(pending) >= 2:
                        flush_fold(pending.pop(0), False)
                stage_c(*post.pop(0))
                flush_fold(pending.pop(0), False)
                flush_fold(pending.pop(0), True)

                s_loc = slocp.tile([128, 512], F32, tag="s_loc")
                nc.scalar.copy(s_loc[:], ps_s[:])
                nc.sync.dma_start(s_in[r][:], s_loc[:])
                nc.gpsimd.collective_compute(
                    "AllReduce", ADD,
                    replica_groups=[list(range(CORES))],
                    ins=[s_in[r].ap().opt()], outs=[s_out[r].ap().opt()],
                )
                s4 = s4p.tile([128, ND], F32, tag="s4")
                for g4 in range(4):
                    nc.sync.dma_start(
                        s4[g4 * 32:(g4 + 1) * 32, :].rearrange(
                            "b (q f) -> b q f", f=512),
                        s_out[r][:].rearrange("(q b) f -> b q f", q=4))
                if r < 2:
                    vb = _squash_build(nc, vbp, sp, tmpp, s4, eps_t[:], 1.0,
                                       BF16)
                else:
                    o32 = _squash_build(nc, slocp, sp, tmpp, s4, eps_t[:], 1.0,
                                        F32, rows=32, tag="o32")
                    nc.sync.dma_start(
                        out[:].rearrange("b n d -> b (n d)"), o32[:])

    nc.compile()
    return nc


_NC_CACHE = {}


def _get_nc():
    if "nc" not in _NC_CACHE:
        _NC_CACHE["nc"] = build_kernel()
    return _NC_CACHE["nc"]


def _make_in_maps(inputs, W):
    inputs = np.ascontiguousarray(np.asarray(inputs, dtype=np.float32))
    W = np.ascontiguousarray(np.asarray(W, dtype=np.float32))
    assert inputs.shape == (B, I, J) and W.shape == (N, I, D, J)
    def pmajor(a):
        # [(k p), cols] row-major -> [p, k*cols] partition-major
        cols = a.shape[1]
        return np.ascontiguousarray(
            a.reshape(CHUNKS, 128, cols).transpose(1, 0, 2).reshape(
                128, CHUNKS * cols))

    in_maps = []
    for c in range(CORES):
        sl = slice(c * I_LOC, (c + 1) * I_LOC)
        # x_t: [(i j), b]
        x_t = inputs[:, sl, :].transpose(1, 2, 0).reshape(
            I_LOC * J, B).astype(ml_dtypes.bfloat16)
        # w_t: [(i j), (n d)] ; w_t[(i,j),(n,d)] = W[n, i, d, j]
        w_t = W[:, sl, :, :].transpose(1, 3, 0, 2).reshape(
            I_LOC * J, ND).astype(ml_dtypes.bfloat16)
        # block-diagonal x: xbd[(i,j), (i4, b)] = x_t[(i,j), b] iff i%4==i4
        xbd = np.zeros((GROUPS, 4, J, 4, B), dtype=ml_dtypes.bfloat16)
        xv = x_t.reshape(GROUPS, 4, J, B)
        for i4 in range(4):
            xbd[:, i4, :, i4, :] = xv[:, i4]
        xbd = xbd.reshape(I_LOC * J, 128)
        in_maps.append({"xt": pmajor(x_t),
                        "xbdt": pmajor(xbd),
                        "wt": pmajor(w_t)})
    return in_maps


def _ensure_ntff_hook():
    """Register the axon NTFF profile hook if the image's antenv lacks it."""
    import types

    try:
        import antenv.axon_hooks  # noqa: F401
        return
    except ImportError:
        pass
    import antenv

    if "/root/.axon_site" not in sys.path:
        sys.path.insert(0, "/root/.axon_site")
    from trn_agent_boot.trn_boot import _ntff_profile_via_ctypes

    hook = {"h": _ntff_profile_via_ctypes("/opt/axon/libaxon_pjrt.so")}
    mod = types.ModuleType("antenv.axon_hooks")
    mod.get_axon_ntff_profile_hook = lambda: hook["h"]
    mod.set_axon_ntff_profile_hook = lambda h: hook.__setitem__("h", h)
    sys.modules["antenv.axon_hooks"] = mod
    antenv.axon_hooks = mod


def run(inputs, W, trace=False):
    nc = _get_nc()
    if trace:
        _ensure_ntff_hook()
        # zero-egress container: skip the artifact upload, keep files local
        import concourse.bass_utils as bu
        bu.upload_artifacts = lambda d: d
    res = run_bass_kernel_spmd(
        nc, _make_in_maps(inputs, W), core_ids=list(range(CORES)),
        trace=trace,
    )
    return res.results[0]["out"].reshape(B, N, D), res


def kernel(inputs, W, routings=R, **_unused):
    assert int(routings) == R
    out, _ = run(inputs, W, trace=False)
    return out


# revision 32
# speedup vs baseline: 1.6843x; 1.1290x over previous
"""CapsuleLayer dynamic-routing kernel for Trainium2 (8 NeuronCores).

Problem: inputs [B=32, I=2048, J=16], W [N=64, I=2048, D=32, J=16], routings=3.
  inputs_hat[b,n,i,d] = sum_j inputs[b,i,j] * W[n,i,d,j]
  3 rounds of routing (softmax over n, weighted sum over i, squash over d).

Strategy: shard the input-capsule axis I across the 8 cores (I_loc=256).
W is cast to bf16 on host and resides wholly in SBUF (16.75 MB), loaded
once. Each core recomputes its ihat shard from W every round; ihat never
touches DRAM. Only cross-core traffic is the [B, N, D] partial sum s,
AllReduced (256 KB) once per round.

Matmuls run in single-product bf16 (harness gate is 2e-2 rel err).
H-compute uses a block-diagonal packing: 4 capsules share one K=64
matmul with M=128 output partitions (4i x 32b), i.e. guaranteed 4x PE
utilization vs per-capsule K=16 matmuls.

On-chip layout per round, per group of 4 input capsules i:
  PE:  4x matmul [K=64=(4i,16j), M=128=(4i,32b), N=512] -> H [128, (n,d)]
  ACT: H psum -> SBUF bf16
  DVE: tmp = H*vb (bf16 2x); y = reduce_d(tmp); b += y;
       e = exp(b) (ACT, accum Z); cg2 = e/Z duplicated in pairs;
       tmp2 = H*c via pair-broadcast AP (bf16 2x, partly on GpSimd)
  PE:  s_psum += sel.T @ tmp2  (folds partition groups AND sums over i)
"""

import sys

for p in ("/opt/trn_rl_repo",):
    if p not in sys.path:
        sys.path.insert(0, p)

import ml_dtypes
import numpy as np

import concourse.bacc as bacc
import concourse.mybir as mybir
import concourse.tile as tile
from concourse.bass_utils import run_bass_kernel_spmd

# problem constants (hardcoded per harness contract)
B, N, I, D, J = 32, 64, 2048, 32, 16
R = 3  # routings
CORES = 8
I_LOC = I // CORES  # 256
ND = N * D  # 2048
EPS = 1e-7

F32 = mybir.dt.float32
BF16 = mybir.dt.bfloat16
FX = mybir.AxisListType.X
ADD = mybir.AluOpType.add
ACT = mybir.ActivationFunctionType

GROUPS = I_LOC // 4  # 64 groups of 4 capsules per round
CHUNKS = I_LOC * J // 128  # 32 row-chunks of the [(i j), *] operands


def _squash_build(nc, vbpool, smalls, kp, s4, eps_ap, scale0, out_dtype,
                  rows=128, tag="sq_vb"):
    """s4: [128, 2048] f32 tile holding raw s (replicated x4 on partition
    groups); real s = scale0 * s4. Returns [rows, 2048] squash(s) tile."""
    # sq = sum_d (scale0*s)^2  (Square activation applies scale before func)
    sq = smalls.tile([128, N], F32, tag="sq_sq")
    s2 = kp.tile([128, ND], BF16, tag="tmp")
    nc.scalar.activation(s2[:], s4[:], ACT.Square, scale=float(scale0))
    nc.vector.tensor_reduce(
        sq[:], s2[:].rearrange("p (n d) -> p n d", d=D), axis=FX, op=ADD)
    # t = sqrt(sq + eps)
    t = smalls.tile([128, N], F32, tag="sq_t")
    nc.scalar.activation(t[:], sq[:], ACT.Sqrt, bias=eps_ap)
    # q1 = 1 + sq
    q1 = smalls.tile([128, N], F32, tag="sq_q1")
    nc.scalar.activation(q1[:], sq[:], ACT.Identity, bias=1.0)
    den = smalls.tile([128, N], F32, tag="sq_den")
    nc.vector.tensor_mul(den[:], q1[:], t[:])
    rs = smalls.tile([128, N], F32, tag="sq_rs")
    nc.vector.reciprocal(rs[:], den[:])
    # scale = sq * rs * scale0   (the last scale0 from s = scale0*raw)
    scale = smalls.tile([128, N], F32, tag="sq_scale")
    nc.vector.tensor_mul(scale[:], sq[:], rs[:])
    if scale0 != 1.0:
        nc.vector.tensor_scalar_mul(scale[:], scale[:], float(scale0))
    vb = vbpool.tile([rows, ND], out_dtype, tag=tag)
    nc.vector.tensor_mul(
        vb[:].rearrange("p (n d) -> p n d", d=D),
        s4[0:rows].rearrange("p (n d) -> p n d", d=D),
        scale[0:rows, :, None].broadcast_to([rows, N, D]),
    )
    return vb


def build_kernel():
    nc = bacc.Bacc("TRN2", target_bir_lowering=False, debug=False)

    # all three staged in partition-major layout: [128, chunks * cols] so a
    # partition-slice DMA is one contiguous run per partition
    xt = nc.dram_tensor("xt", [128, CHUNKS * B], BF16, kind="ExternalInput")
    xbdt = nc.dram_tensor("xbdt", [128, CHUNKS * 128], BF16,
                          kind="ExternalInput")
    wt = nc.dram_tensor("wt", [128, CHUNKS * ND], BF16, kind="ExternalInput")
    out = nc.dram_tensor("out", [B, N, D], F32, kind="ExternalOutput")

    # collective bounce buffers (one pair per round), stored in the fold's
    # col-tiled layout: row 32q+b holds s[b, 512q:512(q+1)]
    s_in = [nc.dram_tensor(f"s_in{r}", [128, 512], BF16) for r in range(R)]
    s_out = [nc.dram_tensor(f"s_out{r}", [128, 512], BF16,
                            addr_space="Shared")
             for r in range(R)]

    with tile.TileContext(nc) as tc:
        with (
            tc.tile_pool(name="persist", bufs=1) as pp,
            tc.tile_pool(name="vbp", bufs=1) as vbp,
            tc.tile_pool(name="s4p", bufs=1) as s4p,
            tc.tile_pool(name="slocp", bufs=1) as slocp,
            tc.tile_pool(name="hsbp", bufs=3) as hsbp,
            tc.tile_pool(name="tmpp", bufs=1) as tmpp,
            tc.tile_pool(name="t2p", bufs=2) as t2p,
            tc.tile_pool(name="redp", bufs=1) as redp,
            tc.tile_pool(name="small", bufs=2) as sp,
            tc.tile_pool(name="psum", bufs=3, space="PSUM") as pgp,
            tc.tile_pool(name="psumB", bufs=1, space="PSUM") as psS,
        ):
            # ---- resident tiles ----
            # W, chunked rows: wsb[p, k, f] = W row 128k+p (partition-major
            # in DRAM; 16 partition-slice DMAs, one contiguous run each)
            # (chunk-wise DMAs: full 128-partition width per transfer — DMA
            # throughput needs wide partition fan-out, 4KB runs/partition)
            # x chunks for round-0 dense einsum (issued before the W
            # stream so round 0 can start on chunk 0 immediately)
            xsb = pp.tile([128, CHUNKS, B], BF16, tag="xsb")
            nc.sync.dma_start(
                xsb[:], xt[:].rearrange("p (k b) -> p k b", b=B))
            # block-diagonal x for rounds 1-2
            xbd = pp.tile([128, CHUNKS, 128], BF16, tag="xbd")
            for u in range(4):
                nc.sync.dma_start(
                    xbd[:, u * 8:(u + 1) * 8, :],
                    xbdt[:, u * 1024:(u + 1) * 1024].rearrange(
                        "p (k m) -> p k m", m=128))
            wsb = pp.tile([128, CHUNKS, ND], BF16, tag="wsb")
            for k in range(CHUNKS):
                nc.sync.dma_start(
                    wsb[:, k, :], wt[:, k * ND:(k + 1) * ND])

            # routing state: exp(b) per group, bf16 [128=(4i,32b), GROUPS, N]
            estate = pp.tile([128, GROUPS, N], BF16, tag="estate")
            eps_t = pp.tile([128, 1], F32, tag="eps")
            nc.gpsimd.memset(eps_t[:], EPS)
            # selector[p, m] = 1.0 if p % 32 == m  (partition-group fold)
            sel_i = pp.tile([128, B], mybir.dt.int32, tag="sel_i")
            nc.gpsimd.iota(sel_i[:], [[1, B]], channel_multiplier=-1)
            nc.vector.tensor_scalar(sel_i[:], sel_i[:], 31, None,
                                    op0=mybir.AluOpType.bitwise_and)
            sel = pp.tile([128, B], BF16, tag="sel")
            nc.vector.tensor_scalar(sel[:], sel_i[:], 0, None,
                                    op0=mybir.AluOpType.is_equal)

            # ---------- round 0: c uniform -> s0 = (1/N) sum_i ihat ----------
            # quarter q of (n,d) accumulates on partitions 32q..32q+32
            # (4-way col-tiled: one PSUM bank, concurrent quarter streams)
            ps0 = psS.tile([128, 512], F32, tag="ps_s")
            for k in range(CHUNKS):
                for q in range(4):
                    nc.tensor.matmul(
                        ps0[32 * q:32 * (q + 1), :],
                        xsb[:, k, :],
                        wsb[:, k, q * 512:(q + 1) * 512],
                        start=(k == 0), stop=(k == CHUNKS - 1),
                        tile_position=(0, 32 * q),
                        skip_group_check=True,
                    )
            s_loc0 = slocp.tile([128, 512], BF16, tag="s_loc")
            nc.scalar.copy(s_loc0[:], ps0[:])
            nc.sync.dma_start(s_in[0][:], s_loc0[:])
            nc.gpsimd.collective_compute(
                "AllReduce", ADD,
                replica_groups=[list(range(CORES))],
                ins=[s_in[0].ap().opt()], outs=[s_out[0].ap().opt()],
            )
            s4 = s4p.tile([128, ND], BF16, tag="s4")
            for g4 in range(4):
                nc.sync.dma_start(
                    s4[g4 * 32:(g4 + 1) * 32, :].rearrange(
                        "b (q f) -> b q f", f=512),
                    s_out[0][:].rearrange("(q b) f -> b q f", q=4))
            vb = _squash_build(nc, vbp, sp, tmpp, s4, eps_t[:], 1.0 / N, BF16)

            # ---------- rounds 1, 2 ----------
            for r in (1, 2):
                ps_s = psS.tile([128, 512], F32, tag="ps_s")
                pending = []  # (g, tmp2) awaiting fold
                post = []     # (g, hsb) awaiting softmax/tmp2

                def flush_fold(pend, last, _ps=ps_s):
                    g0, t2 = pend
                    for q in range(4):
                        nc.tensor.matmul(
                            _ps[32 * q:32 * (q + 1), :],
                            sel[:],
                            t2[:, q * 512:(q + 1) * 512],
                            start=(g0 == 0),
                            stop=(last and q == 3),
                            tile_position=(0, 32 * q),
                            skip_group_check=True,
                        )

                def stage_c(g, y, hsb, _r=r):
                    # softmax over n via running exp(b) state (|b| is O(1):
                    # no max-subtraction needed). estate[g] = prod exp(y_r).
                    esl = estate[:, g, :]
                    zz = sp.tile([128, 1], F32, tag="zz")
                    if _r == 1:
                        nc.scalar.activation(esl, y[:], ACT.Exp,
                                             accum_out=zz[:])
                    else:
                        e2 = sp.tile([128, N], BF16, tag="e2")
                        nc.scalar.activation(e2[:], y[:], ACT.Exp)
                        # GpSimd: tiny op, keeps it off the busy DVE
                        nc.gpsimd.tensor_mul(esl, esl, e2[:])
                        with nc.allow_low_precision("softmax denom in f32"):
                            nc.vector.tensor_reduce(zz[:], esl, axis=FX,
                                                    op=ADD)
                    rz = sp.tile([128, 1], F32, tag="rz")
                    nc.vector.reciprocal(rz[:], zz[:])
                    # cg2[p, n, 0] = cg2[p, n, 1] = e[p, n] / Z[p]
                    # (ScalarE: activation Copy with per-partition scale)
                    cg2 = sp.tile([128, N, 2], BF16, tag="cg2")
                    nc.scalar.activation(
                        cg2[:],
                        estate[:, g, :, None].broadcast_to([128, N, 2]),
                        ACT.Copy, scale=rz[:])
                    # tmp2 = c * H  via pair-packed broadcast (keeps DVE 2x)
                    tmp2 = t2p.tile([128, ND], BF16, tag="tmp2")
                    nc.vector.tensor_mul(
                        tmp2[:].rearrange(
                            "p (n dp two) -> p n dp two", dp=16, two=2),
                        hsb[:].rearrange(
                            "p (n dp two) -> p n dp two", dp=16, two=2),
                        cg2[:, :, None, :].broadcast_to([128, N, 16, 2]),
                    )
                    pending.append((g, tmp2))

                for g in range(GROUPS):
                    half = 64 * (g % 2)
                    chunk = g // 2
                    pg0 = pgp.tile([128, ND // 2], F32, tag="pg")
                    pg1 = pgp.tile([128, ND // 2], F32, tag="pg")
                    for h, pg in ((0, pg0), (1, pg1)):
                        for q in range(2):
                            f0 = h * 1024 + q * 512
                            nc.tensor.matmul(
                                pg[:, q * 512:(q + 1) * 512],
                                xbd[half:half + 64, chunk, :],
                                wsb[half:half + 64, chunk, f0:f0 + 512],
                                start=True, stop=True,
                            )
                    hsb = hsbp.tile([128, ND], BF16, tag="hsb")
                    nc.scalar.copy(hsb[:, :1024], pg0[:])
                    nc.scalar.copy(hsb[:, 1024:], pg1[:])
                    # y = sum_d H * v   (tree: 2x-mode halving adds, then a
                    # short 1x reduce — a direct 2048-wide reduce runs 1x)
                    tmp = tmpp.tile([128, ND], BF16, tag="tmp")
                    nc.vector.tensor_mul(tmp[:], hsb[:], vb[:])
                    t3 = tmp[:].rearrange("p (n d) -> p n d", d=D)
                    tb = redp.tile([128, N, D // 2], BF16, tag="tb")
                    nc.vector.tensor_add(tb[:], t3[:, :, 0:16], t3[:, :, 16:32])
                    tct = redp.tile([128, N, D // 4], BF16, tag="tc")
                    nc.vector.tensor_add(tct[:], tb[:, :, 0:8], tb[:, :, 8:16])
                    y = sp.tile([128, N], BF16, tag="y")
                    with nc.allow_low_precision("bf16 routing logits"):
                        nc.vector.tensor_reduce(y[:], tct[:], axis=FX, op=ADD)
                    post.append((g, y, hsb))
                    if len(post) >= 2:
                        stage_c(*post.pop(0))
                    if len(pending) >= 2:
                        flush_fold(pending.pop(0), False)
                stage_c(*post.pop(0))
                flush_fold(pending.pop(0), False)
                flush_fold(pending.pop(0), True)

                s_loc = slocp.tile([128, 512], BF16, tag="s_loc")
                nc.scalar.copy(s_loc[:], ps_s[:])
                nc.sync.dma_start(s_in[r][:], s_loc[:])
                nc.gpsimd.collective_compute(
                    "AllReduce", ADD,
                    replica_groups=[list(range(CORES))],
                    ins=[s_in[r].ap().opt()], outs=[s_out[r].ap().opt()],
                )
                s4 = s4p.tile([128, ND], BF16, tag="s4")
                for g4 in range(4):
                    nc.sync.dma_start(
                        s4[g4 * 32:(g4 + 1) * 32, :].rearrange(
                            "b (q f) -> b q f", f=512),
                        s_out[r][:].rearrange("(q b) f -> b q f", q=4))
                if r < 2:
                    vb = _squash_build(nc, vbp, sp, tmpp, s4, eps_t[:], 1.0,
                                       BF16)
                else:
                    o32 = _squash_build(nc, slocp, sp, tmpp, s4, eps_t[:], 1.0,
                                        F32, rows=32, tag="o32")
                    nc.sync.dma_start(
                        out[:].rearrange("b n d -> b (n d)"), o32[:])

    nc.compile()
    return nc


_NC_CACHE = {}


def _get_nc():
    if "nc" not in _NC_CACHE:
        _NC_CACHE["nc"] = build_kernel()
    return _NC_CACHE["nc"]


def _make_in_maps(inputs, W):
    inputs = np.ascontiguousarray(np.asarray(inputs, dtype=np.float32))
    W = np.ascontiguousarray(np.asarray(W, dtype=np.float32))
    assert inputs.shape == (B, I, J) and W.shape == (N, I, D, J)
    def pmajor(a):
        # [(k p), cols] row-major -> [p, k*cols] partition-major
        cols = a.shape[1]
        return np.ascontiguousarray(
            a.reshape(CHUNKS, 128, cols).transpose(1, 0, 2).reshape(
                128, CHUNKS * cols))

    in_maps = []
    for c in range(CORES):
        sl = slice(c * I_LOC, (c + 1) * I_LOC)
        # x_t: [(i j), b]
        x_t = inputs[:, sl, :].transpose(1, 2, 0).reshape(
            I_LOC * J, B).astype(ml_dtypes.bfloat16)
        # w_t: [(i j), (n d)] ; w_t[(i,j),(n,d)] = W[n, i, d, j]
        w_t = W[:, sl, :, :].transpose(1, 3, 0, 2).reshape(
            I_LOC * J, ND).astype(ml_dtypes.bfloat16)
        # block-diagonal x: xbd[(i,j), (i4, b)] = x_t[(i,j), b] iff i%4==i4
        xbd = np.zeros((GROUPS, 4, J, 4, B), dtype=ml_dtypes.bfloat16)
        xv = x_t.reshape(GROUPS, 4, J, B)
        for i4 in range(4):
            xbd[:, i4, :, i4, :] = xv[:, i4]
        xbd = xbd.reshape(I_LOC * J, 128)
        in_maps.append({"xt": pmajor(x_t),
                        "xbdt": pmajor(xbd),
                        "wt": pmajor(w_t)})
    return in_maps


def _ensure_ntff_hook():
    """Register the axon NTFF profile hook if the image's antenv lacks it."""
    import types

    try:
        import antenv.axon_hooks  # noqa: F401
        return
    except ImportError:
        pass
    import antenv

    if "/root/.axon_site" not in sys.path:
        sys.path.insert(0, "/root/.axon_site")
    from trn_agent_boot.trn_boot import _ntff_profile_via_ctypes

    hook = {"h": _ntff_profile_via_ctypes("/opt/axon/libaxon_pjrt.so")}
    mod = types.ModuleType("antenv.axon_hooks")
    mod.get_axon_ntff_profile_hook = lambda: hook["h"]
    mod.set_axon_ntff_profile_hook = lambda h: hook.__setitem__("h", h)
    sys.modules["antenv.axon_hooks"] = mod
    antenv.axon_hooks = mod


def run(inputs, W, trace=False):
    nc = _get_nc()
    if trace:
        _ensure_ntff_hook()
        # zero-egress container: skip the artifact upload, keep files local
        import concourse.bass_utils as bu
        bu.upload_artifacts = lambda d: d
    res = run_bass_kernel_spmd(
        nc, _make_in_maps(inputs, W), core_ids=list(range(CORES)),
        trace=trace,
    )
    return res.results[0]["out"].reshape(B, N, D), res


def kernel(inputs, W, routings=R, **_unused):
    assert int(routings) == R
    out, _ = run(inputs, W, trace=False)
    return out
